# revision 24
# baseline (speedup 1.0000x reference)
"""Cross-modal attention kernel for Trainium2, 8 NeuronCores.

Problem (nn_CrossModalAttention): B=2, N=2048, DIM=768, HEADS=12, HD=64.
  q/k/v = Linear(x{1,2}); attn blend: a1 = softmax((1-s)*q1k1 + s*q1k2),
  a2 = softmax((1-s)*q2k2 + s*q2k1); out = (a@v) @ Wo^T + bo.

Key algebraic folds (host side):
  - (1-s)*q1k1 + s*q1k2 = q1 @ ((1-s)k1 + s*k2)^T and k is linear in x, so
    kb1 = ((1-s)x1 + s*x2) @ Wk^T + bk.  Two standard attentions remain.
  - softmax scale folded into Wq/bq.

Sharding: 8 cores = 2 (batch) x 2 (modality) x 2 (head halves of 6 heads).
Each core computes a partial output projection over its 6 heads; host sums
the two head-half partials and adds bo.

Device-side per core (all matmuls in float32r: full PE rate, ~1e-4 rounding):
  - PE-transpose xb_loc -> xbT ([c, n]), project kbT; then x_loc -> xT
    (same SBUF slot), project v (native) and qT per head pair.
  - Attention interleaved with the qT pair projections. Per pair:
    scoresT = kbT_h-slices.T @ qT (row-packed 2 heads per 128-row PE array),
    exp on ACT (no max subtraction: |scores| small), oT65 = [v_h|1].T @ expS
    accumulated over 16 key tiles (row 64 = softmax denominators),
    normalize with DVE reciprocal + gpsimd partition-broadcast.
  - Partial out = oT.T @ w_o, DMA out.
"""

import os
import sys

for _p in ("/opt/trn_rl_repo", "/root/.axon_site/_ro/trn_rl_repo"):
    if os.path.isdir(_p) and _p not in sys.path:
        sys.path.insert(0, _p)

import numpy as np
import ml_dtypes

import concourse.bass as bass
import concourse.tile as tile
from concourse import bacc, mybir
from concourse.bass_utils import run_bass_kernel_spmd
from concourse.masks import make_identity

F32 = mybir.dt.float32
F32R = mybir.dt.float32r
BF16 = mybir.dt.bfloat16
AF = mybir.ActivationFunctionType

# Problem constants
B = 2
NQ = 2048  # sequence length
C = 768  # model dim
HD = 64  # head dim
HL = 6  # heads per core (half of 12)
DL = HL * HD  # 384 local head dims
P = 128
NT = NQ // P  # 16 n tiles
CT = C // P  # 6 contraction tiles
DT = DL // P  # 3 local d tiles (= head pairs)
QH = 1024  # q block (PSUM budget)
SCALE = HD ** -0.5

_cache = {}


def _build_program():
    nc = bacc.Bacc("TRN2", target_bir_lowering=False, debug=False, num_devices=8)

    ext = {
        "x": nc.declare_dram_parameter("x_loc", [NQ, C], F32, isOutput=False),
        "xb": nc.declare_dram_parameter("xb_loc", [NQ, C], F32, isOutput=False),
        "wq": nc.declare_dram_parameter("w_q", [C, DL], F32R, isOutput=False),
        "wk": nc.declare_dram_parameter("w_k", [C, DL], F32R, isOutput=False),
        "wv": nc.declare_dram_parameter("w_v", [C, DL], F32R, isOutput=False),
        "wo": nc.declare_dram_parameter("w_o", [DL, C], F32R, isOutput=False),
        "bq": nc.declare_dram_parameter("b_q", [DL], F32, isOutput=False),
        "bk": nc.declare_dram_parameter("b_k", [DL], F32, isOutput=False),
        "bv": nc.declare_dram_parameter("b_v", [HL * (HD + 1)], F32,
                                        isOutput=False),
        "part": nc.declare_dram_parameter("part", [NQ, C], F32, isOutput=True),
    }
    with tile.TileContext(nc) as tc:
        _trace(nc, tc, ext)
    nc.compile()
    return nc


def _trace(nc, tc, ext):
    from contextlib import ExitStack

    with ExitStack() as es:
        consts = es.enter_context(tc.tile_pool(name="consts", bufs=1))
        qkv_pool = es.enter_context(tc.tile_pool(name="qkv", bufs=1))
        wo_pool = es.enter_context(tc.tile_pool(name="wo", bufs=1))
        oT_pool = es.enter_context(tc.tile_pool(name="oT", bufs=1))
        mx_ps = {}  # psum pools, phase-scoped below

        ident = consts.tile([P, P], F32)
        make_identity(nc, ident[:])
        bq_sb = consts.tile([P, DT], F32)
        nc.sync.dma_start(bq_sb[:], ext["bq"][:].rearrange("(t p) -> p t", p=P))
        bk_sb = consts.tile([P, DT], F32)
        nc.sync.dma_start(bk_sb[:], ext["bk"][:].rearrange("(t p) -> p t", p=P))
        bv_sb = consts.tile([P, HL * (HD + 1)], F32)
        nc.sync.dma_start(bv_sb[:], ext["bv"][:].partition_broadcast(P))
        wo_sb = wo_pool.tile([P, DT, C], F32R)

        qT = qkv_pool.tile([P, DT, NQ], F32R)
        kbT = qkv_pool.tile([P, DT, NQ], F32R)
        v65 = qkv_pool.tile([P, NT, HL * (HD + 1)], F32R)
        oT = oT_pool.tile([P, DT, NQ], F32R)

        def transpose_in(xnat_pool, src_ext, xT_tile):
            # src [n, c] -> xT [c-part, ct, n]
            for nt in range(NT):
                x_nat = xnat_pool.tile([P, C], F32, tag="xnat", name=f"xn{nt}")
                nc.sync.dma_start(x_nat[:], src_ext[nt * P:(nt + 1) * P, :])
                ps = mx_ps["ab"].tile([P, CT * P], F32, tag="tp", name=f"tp{nt}")
                for ct in range(CT):
                    nc.tensor.transpose(
                        ps[:, ct * P:(ct + 1) * P],
                        x_nat[:, ct * P:(ct + 1) * P],
                        ident[:],
                    )
                nc.vector.tensor_copy(
                    xT_tile[:, :, nt * P:(nt + 1) * P],
                    ps[:].rearrange("p (ct x) -> p ct x", x=P),
                )

        def proj_T_dt(w_sb, b_sb, xT_tile, out_tile, dt):
            # out[d-part, dt, n] = w[:, dt-slice].T @ xT (+ per-partition bias)
            for nq in range(NQ // 512):
                ps = mx_ps["ab"].tile([P, 512], F32, tag="pj", name=f"pj{dt}_{nq}")
                for ct in range(CT):
                    nc.tensor.matmul(
                        ps[:],
                        w_sb[:, ct, dt * P:(dt + 1) * P],
                        xT_tile[:, ct, nq * 512:(nq + 1) * 512],
                        start=(ct == 0),
                        stop=(ct == CT - 1),
                    )
                nc.vector.tensor_scalar_add(
                    out_tile[:, dt, nq * 512:(nq + 1) * 512],
                    ps[:],
                    b_sb[:, dt:dt + 1],
                )

        def attention_pair(pair, expp, nrm):
            for qh in range(NQ // QH):
                qs = qh * QH
                avs = [mx_ps["av"].tile([HD + 1, QH], F32, tag=f"av{h}",
                                       name=f"av{pair}_{qh}_{h}") for h in range(2)]
                for krt in range(NT):
                    sps = [mx_ps["sc"].tile([P, QH], F32, tag="sc",
                                            name=f"sc{pair}_{qh}_{krt}_{h}")
                           for h in range(2)]
                    for h in range(2):
                        lo = h * HD
                        for cq in range(QH // 512):
                            nc.tensor.matmul(
                                sps[h][:, cq * 512:(cq + 1) * 512],
                                kbT[lo:lo + HD, pair, krt * P:(krt + 1) * P],
                                qT[lo:lo + HD, pair,
                                   qs + cq * 512:qs + (cq + 1) * 512],
                                start=True,
                                stop=True,
                            )
                    for h in range(2):
                        es_t = expp.tile([P, QH], F32R, tag="expS",
                                         name=f"es{pair}_{qh}_{krt}_{h}")
                        nc.scalar.activation(es_t[:], sps[h][:], AF.Exp)
                        for cq in range(QH // 512):
                            nc.tensor.matmul(
                                avs[h][:, cq * 512:(cq + 1) * 512],
                                v65[:, krt, (pair * 2 + h) * (HD + 1):
                                    (pair * 2 + h + 1) * (HD + 1)],
                                es_t[:, cq * 512:(cq + 1) * 512],
                                start=(krt == 0),
                                stop=(krt == NT - 1),
                            )
                # normalize: oT = av[0:64] * (1/av[64]) broadcast to partitions
                for h in range(2):
                    for cq in range(QH // 512):
                        sl = slice(cq * 512, (cq + 1) * 512)
                        rec = nrm.tile([1, 512], F32, tag="rec",
                                       name=f"rc{pair}_{qh}_{h}_{cq}")
                        nc.vector.reciprocal(rec[:], avs[h][HD:HD + 1, sl])
                        bc = nrm.tile([HD, 512], F32, tag="bc",
                                      name=f"bc{pair}_{qh}_{h}_{cq}")
                        nc.gpsimd.partition_broadcast(bc[:], rec[:])
                        nc.vector.tensor_mul(
                            oT[h * HD:(h + 1) * HD, pair,
                               qs + cq * 512:qs + (cq + 1) * 512],
                            avs[h][0:HD, sl],
                            bc[:],
                        )

        # ---- x first: transpose + v + qT projections ------------------------
        xT_pool = es.enter_context(tc.tile_pool(name="xT", bufs=1))
        psAB = ExitStack()
        mx_ps["ab"] = psAB.enter_context(tc.tile_pool(name="ab_ps", bufs=2, space="PSUM"))
        with ExitStack() as ab1:
            wqv_pool = ab1.enter_context(tc.tile_pool(name="w_qv", bufs=1))
            xn1 = ab1.enter_context(tc.tile_pool(name="xn1", bufs=2))

            wq_sb = wqv_pool.tile([P, CT, DL], F32R, tag="wq")
            nc.sync.dma_start(wq_sb[:],
                              ext["wq"][:].rearrange("(t p) d -> p t d", p=P))
            wv_sb = wqv_pool.tile([P, CT, DL], F32R, tag="wv")
            nc.sync.dma_start(wv_sb[:],
                              ext["wv"][:].rearrange("(t p) d -> p t d", p=P))
            xT = xT_pool.tile([P, CT, NQ], F32R, tag="xT", name="xT")
            transpose_in(xn1, ext["x"], xT)

            for nt in range(NT):
                ps = mx_ps["ab"].tile([P, DL], F32, tag="pv", name=f"pv{nt}")
                for ct in range(CT):
                    nc.tensor.matmul(
                        ps[:],
                        xT[:, ct, nt * P:(nt + 1) * P],
                        wv_sb[:, ct, :],
                        start=(ct == 0),
                        stop=(ct == CT - 1),
                    )
                vv = v65[:, nt, :].rearrange("p (h x) -> p h x", x=HD + 1)
                bvv = bv_sb[:].rearrange("p (h x) -> p h x", x=HD + 1)
                nc.vector.tensor_add(
                    vv[:, :, 0:HD],
                    ps[:].rearrange("p (h x) -> p h x", x=HD),
                    bvv[:, :, 0:HD],
                )
                nc.vector.tensor_copy(vv[:, :, HD:HD + 1], bvv[:, :, HD:HD + 1])
            for dt in range(DT):
                proj_T_dt(wq_sb, bq_sb, xT, qT, dt)

        # ---- xb: transpose (same slot) + kbT projection ---------------------
        expp = es.enter_context(tc.tile_pool(name="expp", bufs=3))
        nrm = es.enter_context(tc.tile_pool(name="nrm", bufs=2))
        with ExitStack() as ab2:
            wk_pool = ab2.enter_context(tc.tile_pool(name="w_k", bufs=1))
            xn2 = ab2.enter_context(tc.tile_pool(name="xn2", bufs=2))

            wk_sb = wk_pool.tile([P, CT, DL], F32R, tag="wk")
            nc.sync.dma_start(wk_sb[:],
                              ext["wk"][:].rearrange("(t p) d -> p t d", p=P))
            xbT = xT_pool.tile([P, CT, NQ], F32R, tag="xT", name="xbT")
            transpose_in(xn2, ext["xb"], xbT)
            for dt in range(DT):
                proj_T_dt(wk_sb, bk_sb, xbT, kbT, dt)

        nc.sync.dma_start(wo_sb[:], ext["wo"][:].rearrange("(t p) c -> p t c", p=P))
        psAB.close()
        with ExitStack() as att:
            mx_ps["sc"] = att.enter_context(
                tc.tile_pool(name="sc_ps", bufs=2, space="PSUM"))
            mx_ps["av"] = att.enter_context(
                tc.tile_pool(name="av_ps", bufs=1, space="PSUM"))
            for pair in range(DT):
                attention_pair(pair, expp, nrm)

        # ---- output projection: part = oT.T @ w_o --------------------------
        with ExitStack() as pd:
            o_ps = pd.enter_context(tc.tile_pool(name="o_ps", bufs=2, space="PSUM"))
            out_pool = pd.enter_context(tc.tile_pool(name="outp", bufs=2))
            for nt in range(NT):
                ps = o_ps.tile([P, C], F32, tag="po", name=f"po{nt}")
                for half in range(2):
                    w = 512 if half == 0 else C - 512
                    for dt in range(DT):
                        nc.tensor.matmul(
                            ps[:, half * 512:half * 512 + w],
                            oT[:, dt, nt * P:(nt + 1) * P],
                            wo_sb[:, dt, half * 512:half * 512 + w],
                            start=(dt == 0),
                            stop=(dt == DT - 1),
                        )
                osb = out_pool.tile([P, C], F32, tag="osb", name=f"ob{nt}")
                nc.vector.tensor_copy(osb[:], ps[:])
                nc.sync.dma_start(ext["part"][nt * P:(nt + 1) * P, :], osb[:])


def _prep_in_maps(x1, x2, Wq, bq, Wk, bk, Wv, bv, Wo, bo, cross_scale):
    s = float(np.asarray(cross_scale).reshape(-1)[0])
    xb1 = ((1.0 - s) * x1 + s * x2).astype(np.float32)
    xb2 = ((1.0 - s) * x2 + s * x1).astype(np.float32)
    wq_s = (SCALE * Wq).astype(np.float32)
    bq_s = (SCALE * bq).astype(np.float32)
    xs = (x1, x2)
    xbs = (xb1, xb2)
    in_maps = []
    for core in range(8):
        b, mod, half = core >> 2, (core >> 1) & 1, core & 1
        hs = slice(half * DL, (half + 1) * DL)
        in_maps.append({
            "x_loc": np.ascontiguousarray(xs[mod][b]),
            "xb_loc": np.ascontiguousarray(xbs[mod][b]),
            "w_q": np.ascontiguousarray(wq_s[hs, :].T),
            "w_k": np.ascontiguousarray(Wk[hs, :].T),
            "w_v": np.ascontiguousarray(Wv[hs, :].T),
            "w_o": np.ascontiguousarray(Wo[:, hs].T),
            "b_q": np.ascontiguousarray(bq_s[hs]),
            "b_k": np.ascontiguousarray(bk[hs]),
            "b_v": np.ascontiguousarray(
                np.concatenate([bv[hs].reshape(HL, HD),
                                np.ones((HL, 1), np.float32)], axis=1).reshape(-1)),
        })
    return in_maps


def kernel(x1, x2, Wq, bq, Wk, bk, Wv, bv, Wo, bo, cross_scale, _trace_opts=None):
    args = [np.asarray(a, dtype=np.float32) for a in
            (x1, x2, Wq, bq, Wk, bk, Wv, bv, Wo, bo, cross_scale)]
    x1, x2, Wq, bq, Wk, bk, Wv, bv, Wo, bo, cross_scale = args

    if "nc" not in _cache:
        _cache["nc"] = _build_program()
    nc = _cache["nc"]

    in_maps = _prep_in_maps(x1, x2, Wq, bq, Wk, bk, Wv, bv, Wo, bo, cross_scale)
    res = run_bass_kernel_spmd(nc, in_maps, list(range(8)), **(_trace_opts or {}))
    _cache["last_results"] = res

    out1 = np.empty((B, NQ, C), np.float32)
    out2 = np.empty((B, NQ, C), np.float32)
    outs = (out1, out2)
    for b in range(B):
        for mod in range(2):
            core0 = (b << 2) | (mod << 1)
            outs[mod][b] = (res.results[core0]["part"]
                            + res.results[core0 + 1]["part"] + bo)
    return out1, out2


# revision 27
# speedup vs baseline: 1.0470x; 1.0470x over previous
"""Cross-modal attention kernel for Trainium2, 8 NeuronCores.

Problem (nn_CrossModalAttention): B=2, N=2048, DIM=768, HEADS=12, HD=64.
  q/k/v = Linear(x{1,2}); attn blend: a1 = softmax((1-s)*q1k1 + s*q1k2),
  a2 = softmax((1-s)*q2k2 + s*q2k1); out = (a@v) @ Wo^T + bo.

Key algebraic folds (host side):
  - (1-s)*q1k1 + s*q1k2 = q1 @ ((1-s)k1 + s*k2)^T and k is linear in x, so
    kb1 = ((1-s)x1 + s*x2) @ Wk^T + bk.  Two standard attentions remain.
  - softmax scale folded into Wq/bq.

Sharding: 8 cores = 2 (batch) x 2 (modality) x 2 (head halves of 6 heads).
Each core computes a partial output projection over its 6 heads; host sums
the two head-half partials and adds bo.

Device-side per core (all matmuls in float32r: full PE rate, ~1e-4 rounding):
  - PE-transpose xb_loc -> xbT ([c, n]), project kbT; then x_loc -> xT
    (same SBUF slot), project v (native) and qT per head pair.
  - Attention interleaved with the qT pair projections. Per pair:
    scoresT = kbT_h-slices.T @ qT (row-packed 2 heads per 128-row PE array),
    exp on ACT (no max subtraction: |scores| small), oT65 = [v_h|1].T @ expS
    accumulated over 16 key tiles (row 64 = softmax denominators),
    normalize with DVE reciprocal + gpsimd partition-broadcast.
  - Partial out = oT.T @ w_o, DMA out.
"""

import os
import sys

for _p in ("/opt/trn_rl_repo", "/root/.axon_site/_ro/trn_rl_repo"):
    if os.path.isdir(_p) and _p not in sys.path:
        sys.path.insert(0, _p)

import numpy as np
import ml_dtypes

import concourse.bass as bass
import concourse.tile as tile
from concourse import bacc, mybir
from concourse.bass_utils import run_bass_kernel_spmd
from concourse.masks import make_identity

F32 = mybir.dt.float32
F32R = mybir.dt.float32r
BF16 = mybir.dt.bfloat16
AF = mybir.ActivationFunctionType

# Problem constants
B = 2
NQ = 2048  # sequence length
C = 768  # model dim
HD = 64  # head dim
HL = 6  # heads per core (half of 12)
DL = HL * HD  # 384 local head dims
P = 128
NT = NQ // P  # 16 n tiles
CT = C // P  # 6 contraction tiles
DT = DL // P  # 3 local d tiles (= head pairs)
QH = 1024  # q block (PSUM budget)
SCALE = HD ** -0.5

_cache = {}


def _build_program():
    nc = bacc.Bacc("TRN2", target_bir_lowering=False, debug=False, num_devices=8)

    ext = {
        "x": nc.declare_dram_parameter("x_loc", [NQ, C], F32, isOutput=False),
        "xb": nc.declare_dram_parameter("xb_loc", [NQ, C], F32, isOutput=False),
        "wq": nc.declare_dram_parameter("w_q", [C, DL], F32R, isOutput=False),
        "wk": nc.declare_dram_parameter("w_k", [C, DL], F32R, isOutput=False),
        "wv": nc.declare_dram_parameter("w_v", [C, DL], F32R, isOutput=False),
        "wo": nc.declare_dram_parameter("w_o", [DL, C], F32R, isOutput=False),
        "bq": nc.declare_dram_parameter("b_q", [DL], F32, isOutput=False),
        "bk": nc.declare_dram_parameter("b_k", [DL], F32, isOutput=False),
        "bv": nc.declare_dram_parameter("b_v", [HL * (HD + 1)], F32,
                                        isOutput=False),
        "part": nc.declare_dram_parameter("part", [NQ, C], F32, isOutput=True),
    }
    with tile.TileContext(nc) as tc:
        _trace(nc, tc, ext)
    nc.compile()
    return nc


def _trace(nc, tc, ext):
    from contextlib import ExitStack

    with ExitStack() as es:
        consts = es.enter_context(tc.tile_pool(name="consts", bufs=1))
        qkv_pool = es.enter_context(tc.tile_pool(name="qkv", bufs=1))
        wo_pool = es.enter_context(tc.tile_pool(name="wo", bufs=1))
        oT_pool = es.enter_context(tc.tile_pool(name="oT", bufs=1))
        mx_ps = {}  # psum pools, phase-scoped below

        ident = consts.tile([P, P], F32)
        make_identity(nc, ident[:])
        bq_sb = consts.tile([P, DT], F32)
        nc.sync.dma_start(bq_sb[:], ext["bq"][:].rearrange("(t p) -> p t", p=P))
        bk_sb = consts.tile([P, DT], F32)
        nc.sync.dma_start(bk_sb[:], ext["bk"][:].rearrange("(t p) -> p t", p=P))
        bv_sb = consts.tile([P, HL * (HD + 1)], F32)
        nc.sync.dma_start(bv_sb[:], ext["bv"][:].partition_broadcast(P))
        wo_sb = wo_pool.tile([P, DT, C], F32R)

        qT = qkv_pool.tile([P, DT, NQ], F32R)
        kbT = qkv_pool.tile([P, DT, NQ], F32R)
        v65 = qkv_pool.tile([P, NT, HL * (HD + 1)], F32R)
        oT = oT_pool.tile([P, DT, NQ], F32R)

        def transpose_in(xnat_pool, src_ext, xT_tile):
            # src [n, c] -> xT [c-part, ct, n]
            for nt in range(NT):
                x_nat = xnat_pool.tile([P, C], F32, tag="xnat", name=f"xn{nt}")
                nc.sync.dma_start(x_nat[:], src_ext[nt * P:(nt + 1) * P, :])
                ps = mx_ps["ab"].tile([P, CT * P], F32, tag="tp", name=f"tp{nt}")
                for ct in range(CT):
                    nc.tensor.transpose(
                        ps[:, ct * P:(ct + 1) * P],
                        x_nat[:, ct * P:(ct + 1) * P],
                        ident[:],
                    )
                nc.vector.tensor_copy(
                    xT_tile[:, :, nt * P:(nt + 1) * P],
                    ps[:].rearrange("p (ct x) -> p ct x", x=P),
                )

        def proj_T_dt(w_sb, b_sb, xT_tile, out_tile, dt):
            # out[d-part, dt, n] = w[:, dt-slice].T @ xT (+ per-partition bias)
            for nq in range(NQ // 512):
                ps = mx_ps["ab"].tile([P, 512], F32, tag="pj", name=f"pj{dt}_{nq}")
                for ct in range(CT):
                    nc.tensor.matmul(
                        ps[:],
                        w_sb[:, ct, dt * P:(dt + 1) * P],
                        xT_tile[:, ct, nq * 512:(nq + 1) * 512],
                        start=(ct == 0),
                        stop=(ct == CT - 1),
                    )
                nc.vector.tensor_scalar_add(
                    out_tile[:, dt, nq * 512:(nq + 1) * 512],
                    ps[:],
                    b_sb[:, dt:dt + 1],
                )

        def attention_pair(pair, expp, nrm):
            for qh in range(NQ // QH):
                qs = qh * QH
                avs = [mx_ps["av"].tile([HD + 1, QH], F32, tag=f"av{h}",
                                       name=f"av{pair}_{qh}_{h}") for h in range(2)]
                for krt in range(NT):
                    sps = [mx_ps["sc"].tile([P, QH], F32, tag="sc",
                                            name=f"sc{pair}_{qh}_{krt}_{h}")
                           for h in range(2)]
                    for h in range(2):
                        lo = h * HD
                        for cq in range(QH // 512):
                            nc.tensor.matmul(
                                sps[h][:, cq * 512:(cq + 1) * 512],
                                kbT[lo:lo + HD, pair, krt * P:(krt + 1) * P],
                                qT[lo:lo + HD, pair,
                                   qs + cq * 512:qs + (cq + 1) * 512],
                                start=True,
                                stop=True,
                            )
                    for h in range(2):
                        es_t = expp.tile([P, QH], F32R, tag="expS",
                                         name=f"es{pair}_{qh}_{krt}_{h}")
                        nc.scalar.activation(es_t[:], sps[h][:], AF.Exp)
                        for cq in range(QH // 512):
                            nc.tensor.matmul(
                                avs[h][:, cq * 512:(cq + 1) * 512],
                                v65[:, krt, (pair * 2 + h) * (HD + 1):
                                    (pair * 2 + h + 1) * (HD + 1)],
                                es_t[:, cq * 512:(cq + 1) * 512],
                                start=(krt == 0),
                                stop=(krt == NT - 1),
                            )
                # normalize: oT = av[0:64] * (1/av[64]) broadcast to partitions
                for h in range(2):
                    for cq in range(QH // 512):
                        sl = slice(cq * 512, (cq + 1) * 512)
                        rec = nrm.tile([1, 512], F32, tag="rec",
                                       name=f"rc{pair}_{qh}_{h}_{cq}")
                        nc.vector.reciprocal(rec[:], avs[h][HD:HD + 1, sl])
                        bc = nrm.tile([HD, 512], F32, tag="bc",
                                      name=f"bc{pair}_{qh}_{h}_{cq}")
                        nc.gpsimd.partition_broadcast(bc[:], rec[:])
                        nc.vector.tensor_mul(
                            oT[h * HD:(h + 1) * HD, pair,
                               qs + cq * 512:qs + (cq + 1) * 512],
                            avs[h][0:HD, sl],
                            bc[:],
                        )

        # ---- x first: transpose + v + qT projections ------------------------
        xT_pool = es.enter_context(tc.tile_pool(name="xT", bufs=1))
        psAB = ExitStack()
        mx_ps["ab"] = psAB.enter_context(tc.tile_pool(name="ab_ps", bufs=2, space="PSUM"))
        with ExitStack() as ab1:
            wqv_pool = ab1.enter_context(tc.tile_pool(name="w_qv", bufs=1))
            xn1 = ab1.enter_context(tc.tile_pool(name="xn1", bufs=2))

            wq_sb = wqv_pool.tile([P, CT, DL], F32R, tag="wq")
            nc.sync.dma_start(wq_sb[:],
                              ext["wq"][:].rearrange("(t p) d -> p t d", p=P))
            wv_sb = wqv_pool.tile([P, CT, DL], F32R, tag="wv")
            nc.sync.dma_start(wv_sb[:],
                              ext["wv"][:].rearrange("(t p) d -> p t d", p=P))
            xT = xT_pool.tile([P, CT, NQ], F32R, tag="xT", name="xT")
            transpose_in(xn1, ext["x"], xT)

            for nt in range(NT):
                ps = mx_ps["ab"].tile([P, DL], F32, tag="pv", name=f"pv{nt}")
                for ct in range(CT):
                    nc.tensor.matmul(
                        ps[:],
                        xT[:, ct, nt * P:(nt + 1) * P],
                        wv_sb[:, ct, :],
                        start=(ct == 0),
                        stop=(ct == CT - 1),
                    )
                vv = v65[:, nt, :].rearrange("p (h x) -> p h x", x=HD + 1)
                bvv = bv_sb[:].rearrange("p (h x) -> p h x", x=HD + 1)
                nc.vector.tensor_add(
                    vv[:, :, 0:HD],
                    ps[:].rearrange("p (h x) -> p h x", x=HD),
                    bvv[:, :, 0:HD],
                )
                nc.vector.tensor_copy(vv[:, :, HD:HD + 1], bvv[:, :, HD:HD + 1])
            for dt in range(DT):
                proj_T_dt(wq_sb, bq_sb, xT, qT, dt)

        # ---- xb: transpose (same slot) + kbT projection ---------------------
        expp = es.enter_context(tc.tile_pool(name="expp", bufs=5))
        nrm = es.enter_context(tc.tile_pool(name="nrm", bufs=4))
        with ExitStack() as ab2:
            wk_pool = ab2.enter_context(tc.tile_pool(name="w_k", bufs=1))
            xn2 = ab2.enter_context(tc.tile_pool(name="xn2", bufs=2))

            wk_sb = wk_pool.tile([P, CT, DL], F32R, tag="wk")
            nc.sync.dma_start(wk_sb[:],
                              ext["wk"][:].rearrange("(t p) d -> p t d", p=P))
            xbT = xT_pool.tile([P, CT, NQ], F32R, tag="xT", name="xbT")
            transpose_in(xn2, ext["xb"], xbT)
            for dt in range(DT):
                proj_T_dt(wk_sb, bk_sb, xbT, kbT, dt)

        nc.sync.dma_start(wo_sb[:], ext["wo"][:].rearrange("(t p) c -> p t c", p=P))
        psAB.close()
        with ExitStack() as att:
            mx_ps["sc"] = att.enter_context(
                tc.tile_pool(name="sc_ps", bufs=2, space="PSUM"))
            mx_ps["av"] = att.enter_context(
                tc.tile_pool(name="av_ps", bufs=1, space="PSUM"))
            for pair in range(DT):
                attention_pair(pair, expp, nrm)

        # ---- output projection: part = oT.T @ w_o --------------------------
        with ExitStack() as pd:
            o_ps = pd.enter_context(tc.tile_pool(name="o_ps", bufs=2, space="PSUM"))
            out_pool = pd.enter_context(tc.tile_pool(name="outp", bufs=3))
            for nt in range(NT):
                ps = o_ps.tile([P, C], F32, tag="po", name=f"po{nt}")
                for half in range(2):
                    w = 512 if half == 0 else C - 512
                    for dt in range(DT):
                        nc.tensor.matmul(
                            ps[:, half * 512:half * 512 + w],
                            oT[:, dt, nt * P:(nt + 1) * P],
                            wo_sb[:, dt, half * 512:half * 512 + w],
                            start=(dt == 0),
                            stop=(dt == DT - 1),
                        )
                osb = out_pool.tile([P, C], F32, tag="osb", name=f"ob{nt}")
                nc.vector.tensor_copy(osb[:], ps[:])
                nc.sync.dma_start(ext["part"][nt * P:(nt + 1) * P, :], osb[:])


def _prep_in_maps(x1, x2, Wq, bq, Wk, bk, Wv, bv, Wo, bo, cross_scale):
    s = float(np.asarray(cross_scale).reshape(-1)[0])
    xb1 = ((1.0 - s) * x1 + s * x2).astype(np.float32)
    xb2 = ((1.0 - s) * x2 + s * x1).astype(np.float32)
    wq_s = (SCALE * Wq).astype(np.float32)
    bq_s = (SCALE * bq).astype(np.float32)
    xs = (x1, x2)
    xbs = (xb1, xb2)
    in_maps = []
    for core in range(8):
        b, mod, half = core >> 2, (core >> 1) & 1, core & 1
        hs = slice(half * DL, (half + 1) * DL)
        in_maps.append({
            "x_loc": np.ascontiguousarray(xs[mod][b]),
            "xb_loc": np.ascontiguousarray(xbs[mod][b]),
            "w_q": np.ascontiguousarray(wq_s[hs, :].T),
            "w_k": np.ascontiguousarray(Wk[hs, :].T),
            "w_v": np.ascontiguousarray(Wv[hs, :].T),
            "w_o": np.ascontiguousarray(Wo[:, hs].T),
            "b_q": np.ascontiguousarray(bq_s[hs]),
            "b_k": np.ascontiguousarray(bk[hs]),
            "b_v": np.ascontiguousarray(
                np.concatenate([bv[hs].reshape(HL, HD),
                                np.ones((HL, 1), np.float32)], axis=1).reshape(-1)),
        })
    return in_maps


def kernel(x1, x2, Wq, bq, Wk, bk, Wv, bv, Wo, bo, cross_scale, _trace_opts=None):
    args = [np.asarray(a, dtype=np.float32) for a in
            (x1, x2, Wq, bq, Wk, bk, Wv, bv, Wo, bo, cross_scale)]
    x1, x2, Wq, bq, Wk, bk, Wv, bv, Wo, bo, cross_scale = args

    if "nc" not in _cache:
        _cache["nc"] = _build_program()
    nc = _cache["nc"]

    in_maps = _prep_in_maps(x1, x2, Wq, bq, Wk, bk, Wv, bv, Wo, bo, cross_scale)
    res = run_bass_kernel_spmd(nc, in_maps, list(range(8)), **(_trace_opts or {}))
    _cache["last_results"] = res

    out1 = np.empty((B, NQ, C), np.float32)
    out2 = np.empty((B, NQ, C), np.float32)
    outs = (out1, out2)
    for b in range(B):
        for mod in range(2):
            core0 = (b << 2) | (mod << 1)
            outs[mod][b] = (res.results[core0]["part"]
                            + res.results[core0 + 1]["part"] + bo)
    return out1, out2
